# revision 10
# baseline (speedup 1.0000x reference)
"""Trainium2 Bass kernel for nn_FT_init: pixel_unshuffle -> FFT2 -> all-pairs
magnitude/phase recombination -> IFFT2 -> pixel_shuffle.

Strategy: pure data parallel, one sample per NeuronCore (8 cores).

All matmul operands are fp16 (1 cycle/row at any moving width + fast weight
load eligibility; ~5e-4 per-rounding error vs the 2e-2 gate).  PSUM
accumulation stays fp32.

Forward FFT2: stage A streams 256-col [Wr|Wi] movs per channel; stage B is
half-width packed — only frequency cols n = 0..64 of Z are ever consumed
downstream (every recombined spectrum is Hermitian), so stage B computes
[Zr(0:65)|Zi(0:65)] with 130-col movs and the whole magnitude / unit-phase
pipeline runs at half width.

Inverse (Hermitian-folded): for channel pair (i, j) the spectrum
c = mag_i * u_j is Hermitian, so its IFFT2 is real and stage 1 only needs
packed output rows [S1r(n=0..64) | S1i(n=1..63)] = exactly 128 PSUM
partitions from two 128-col matmuls (stationary = packed c tiles, movings =
Re/-Im IDFT consts).  Stage 2 is a single 128-col matmul per j with one
shared folded const; the pixel-shuffle interleave happens in the PSUM
eviction AP.  PE streaming: 384 cols per (i, j) unit vs 1024 per j-pair in
the f32r scheme.  Evictions are [128,1024] two-bank ops to amortize per-op
overhead; output goes to DRAM as fp16 and is upcast on host.
"""
import sys

sys.path.insert(0, "/opt/trn_rl_repo")

import numpy as np
import concourse.bass as bass  # noqa: E402
import concourse.mybir as mybir  # noqa: E402
import concourse.tile as tile  # noqa: E402
import concourse.bacc as bacc  # noqa: E402
from concourse.bass_utils import run_bass_kernel_spmd  # noqa: E402

F32 = mybir.dt.float32
FP16 = mybir.dt.float16
N = 128  # DFT size
R = 4  # msfa / pixel shuffle factor
C = 16  # channels = R*R
MULT = mybir.AluOpType.mult
ADD = mybir.AluOpType.add
SQRT = mybir.ActivationFunctionType.Sqrt


def _dft_consts():
    k = np.arange(N)
    ang = 2.0 * np.pi / N * np.outer(k, k)
    Wr = np.cos(ang).astype(np.float32)
    Wi = (-np.sin(ang)).astype(np.float32)
    Er = (np.cos(ang) / N).astype(np.float32)
    Ei = (np.sin(ang) / N).astype(np.float32)
    # stage-2 folded const: out[y,x] = sum_k s[k,y] * M2[k,x] with
    # s = [S1r(n=0..64) | S1i(n=1..63)]
    w = np.ones(65, np.float32)
    w[1:64] = 2.0
    M2 = np.zeros((N, N), np.float32)
    M2[0:65] = w[:, None] * Er[0:65]
    M2[65:128] = -2.0 * Ei[1:64]
    cinh = np.hstack([
        Wr, Wi,                      # WWh      [0:256]
        Wr[:, 0:65], Wi[:, 0:65],    # WB1      [256:386]
        -Wi[:, 0:65], Wr[:, 0:65],   # WB2      [386:516]
        Er,                          # Erh      [516:644]
        -Ei,                         # Ginh     [644:772]
        M2,                          # M2h      [772:900]
    ]).astype(np.float16)
    return cinh


REPEAT = 1  # >1 only for timing experiments
TIMING_MODE = False  # True: big output stays on-device (for wall-time diffs)


def _build():
    nc = bacc.Bacc("TRN2", target_bir_lowering=False, debug=False, num_devices=8)
    xin = nc.dram_tensor("xin", [128, 2048], FP16, kind="ExternalInput")
    cinh = nc.dram_tensor("cinh", [128, 900], FP16, kind="ExternalInput")
    if TIMING_MODE:
        outd = nc.dram_tensor("outd", [C, 128, 2048], FP16, kind="Internal")
        tiny = nc.dram_tensor("tiny", [1, 128], F32, kind="ExternalOutput")
    else:
        outd = nc.dram_tensor("outd", [C, 128, 2048], FP16, kind="ExternalOutput")

    with tile.TileContext(nc) as tc:
        with (
            tc.tile_pool(name="persist", bufs=1) as pp,
            tc.tile_pool(name="ew", bufs=2) as ew,
            tc.tile_pool(name="sx", bufs=3) as sxp,
            tc.tile_pool(name="cpool", bufs=2) as cpool,
            tc.tile_pool(name="s4pool", bufs=3) as s4p,
            tc.tile_pool(name="ohpool", bufs=3) as ohp,
            tc.tile_pool(name="ps1", bufs=2, space="PSUM") as ps1,
            tc.tile_pool(name="ps2", bufs=2, space="PSUM") as ps2,
        ):
            consts = pp.tile([128, 900], FP16)
            nc.sync.dma_start(consts[:], cinh[:, :])
            WWh = consts[:, 0:256]
            WB1 = consts[:, 256:386]
            WB2 = consts[:, 386:516]
            Erh = consts[:, 516:644]
            Ginh = consts[:, 644:772]
            M2h = consts[:, 772:900]

            xrows = pp.tile([128, 2048], FP16)
            nc.sync.dma_start(xrows[:], xin[:, :])

            for _rep in range(REPEAT):
                # zs: per channel c, 130 cols: [zr(n=0..64) | zi(n=0..64)]
                zs = pp.tile([128, 2080], F32)

                # ---- forward FFT2 (fp16), 2 channels per group ----
                # software-pipelined: stage-A MMs of group g+1 issue before
                # stage-B MMs of group g so the PE streams through the
                # sx-eviction gaps.
                psA_t = [None] * 8
                sx_t = [None] * 8

                def fwd_A(g):
                    psA = ps1.tile([128, 512], F32, tag="s1")
                    psA_t[g] = psA
                    for cc in range(2):
                        c = g * 2 + cc
                        p, q = divmod(c, R)
                        xs = bass.AP(
                            xrows[:].tensor,
                            xrows[:].offset + p * 512 + q,
                            [xrows[:].ap[0], [4, 128]],
                        )
                        nc.tensor.matmul(psA[:, cc * 256:(cc + 1) * 256], xs, WWh,
                                         start=True, stop=True)
                    sx = sxp.tile([128, 512], FP16)
                    sx_t[g] = sx
                    if g % 2 == 0:
                        nc.scalar.copy(sx[:], psA[:])
                    else:
                        nc.vector.tensor_copy(sx[:], psA[:])

                def fwd_B(g):
                    sx = sx_t[g]
                    psB = ps2.tile([128, 512], F32, tag="s2")
                    for cc in range(2):
                        o = cc * 256
                        zo = cc * 130
                        nc.tensor.matmul(psB[:, zo:zo + 130], sx[:, o:o + 128],
                                         WB1, start=True, stop=False)
                        nc.tensor.matmul(psB[:, zo:zo + 130], sx[:, o + 128:o + 256],
                                         WB2, start=False, stop=True)
                    # evict z: [Zr(0:65)|Zi(0:65)] x 2 chans, contiguous 260
                    if g % 2 == 0:
                        nc.vector.tensor_copy(zs[:, g * 260:(g + 1) * 260],
                                              psB[:, 0:260])
                    else:
                        nc.scalar.copy(zs[:, g * 260:(g + 1) * 260],
                                       psB[:, 0:260])

                for g in range(8):
                    fwd_A(g)
                    if g >= 1:
                        fwd_B(g - 1)
                fwd_B(7)

                # ---- magnitude / unit phase, half width (cols 0..64) ----
                # processed in 2 halves of 8 channels each so the inverse can
                # start while the second half is still in flight.
                t1 = pp.tile([128, 1040], F32)
                t2 = pp.tile([128, 1040], F32)
                sq = pp.tile([128, 1040], F32)
                mag_f = pp.tile([128, 1040], F32)
                rmag = pp.tile([128, 1040], F32)
                rmagn = pp.tile([128, 1040], F32)
                uPA = pp.tile([128, 2048], FP16)
                uPB = pp.tile([128, 2048], FP16)
                magP = pp.tile([128, 2048], FP16)

                def apv(t, off, dims):
                    return bass.AP(t[:].tensor, t[:].offset + off, [t[:].ap[0]] + dims)

                for h in range(2):
                    zo = h * 1040  # 8 chans * 130
                    co = h * 520   # 8 chans * 65
                    uo = h * 1024
                    zr_v = apv(zs, zo, [[130, 8], [1, 65]])
                    zi_v = apv(zs, zo + 65, [[130, 8], [1, 65]])
                    t1v = apv(t1, co, [[65, 8], [1, 65]])
                    t2v = apv(t2, co, [[65, 8], [1, 65]])
                    nc.vector.tensor_tensor(t1v, zr_v, zr_v, MULT)
                    nc.vector.tensor_tensor(t2v, zi_v, zi_v, MULT)
                    nc.vector.tensor_tensor(sq[:, co:co + 520], t1[:, co:co + 520],
                                            t2[:, co:co + 520], ADD)
                    nc.scalar.activation(mag_f[:, co:co + 520], sq[:, co:co + 520],
                                         SQRT)
                    scr = ew.tile([128, 520], F32, tag="scr")
                    nc.vector.reciprocal_approx_accurate(
                        rmag[:, co:co + 520], mag_f[:, co:co + 520], scr[:])
                    nc.vector.tensor_scalar_mul(rmagn[:, co:co + 520],
                                                rmag[:, co:co + 520], -1.0)
                    # packed unit-phase tiles (fp16):
                    # uPA[j-block] = [ur(0:65) | ui(1:64)]
                    # uPB[j-block] = [ui(0:65) | -ur(1:64)]
                    nc.vector.tensor_tensor(
                        apv(uPA, uo, [[128, 8], [1, 65]]), zr_v,
                        apv(rmag, co, [[65, 8], [1, 65]]), MULT)
                    nc.vector.tensor_tensor(
                        apv(uPA, uo + 65, [[128, 8], [1, 63]]),
                        apv(zs, zo + 66, [[130, 8], [1, 63]]),
                        apv(rmag, co + 1, [[65, 8], [1, 63]]), MULT)
                    nc.vector.tensor_tensor(
                        apv(uPB, uo, [[128, 8], [1, 65]]), zi_v,
                        apv(rmag, co, [[65, 8], [1, 65]]), MULT)
                    nc.vector.tensor_tensor(
                        apv(uPB, uo + 65, [[128, 8], [1, 63]]),
                        apv(zs, zo + 1, [[130, 8], [1, 63]]),
                        apv(rmagn, co + 1, [[65, 8], [1, 63]]), MULT)
                    # packed magnitudes per i: [mag(0:65) | mag(1:64)]
                    nc.vector.tensor_copy(
                        apv(magP, uo, [[128, 8], [1, 65]]),
                        apv(mag_f, co, [[65, 8], [1, 65]]))
                    nc.vector.tensor_copy(
                        apv(magP, uo + 65, [[128, 8], [1, 63]]),
                        apv(mag_f, co + 1, [[65, 8], [1, 63]]))

                # ---- inverse: per magnitude channel i ----
                ev = 0
                for i in range(C):
                    mb = bass.AP(magP[:].tensor, magP[:].offset + i * 128,
                                 [magP[:].ap[0], [0, 8], [1, 128]])
                    cA = cpool.tile([128, 2048], FP16, tag="cA")
                    cB = cpool.tile([128, 2048], FP16, tag="cB")

                    for half in range(2):
                        uo = half * 1024
                        cAv = apv(cA, uo, [[128, 8], [1, 128]])
                        cBv = apv(cB, uo, [[128, 8], [1, 128]])
                        uAv = apv(uPA, uo, [[128, 8], [1, 128]])
                        uBv = apv(uPB, uo, [[128, 8], [1, 128]])
                        if i % 8 == 7:
                            nc.gpsimd.tensor_tensor(cAv, uAv, mb, MULT)
                        else:
                            nc.vector.tensor_tensor(cAv, uAv, mb, MULT)
                        if i % 2 == 1:
                            nc.gpsimd.tensor_tensor(cBv, uBv, mb, MULT)
                        else:
                            nc.vector.tensor_tensor(cBv, uBv, mb, MULT)
                        ps1t = ps1.tile([128, 1024], F32, tag="s1")
                        for jj in range(8):
                            j = half * 8 + jj
                            o = jj * 128
                            nc.tensor.matmul(ps1t[:, o:o + 128],
                                             cA[:, j * 128:(j + 1) * 128], Erh,
                                             start=True, stop=False)
                            nc.tensor.matmul(ps1t[:, o:o + 128],
                                             cB[:, j * 128:(j + 1) * 128], Ginh,
                                             start=False, stop=True)
                        s4 = s4p.tile([128, 1024], FP16)
                        ev += 1
                        if ev % 4 == 0:
                            nc.vector.tensor_copy(s4[:], ps1t[:])
                        else:
                            nc.scalar.copy(s4[:], ps1t[:])

                        ps2t = ps2.tile([128, 1024], F32, tag="s2")
                        for jj in range(8):
                            o = jj * 128
                            nc.tensor.matmul(ps2t[:, o:o + 128],
                                             s4[:, jj * 128:(jj + 1) * 128],
                                             M2h, start=True, stop=True)
                        # evict with pixel-shuffle interleave:
                        # oh[:, pp*512 + 4x + jx] = ps2t[:, (pp*4+jx)*128 + x]
                        oh = ohp.tile([128, 1024], FP16)
                        src = bass.AP(ps2t[:].tensor, ps2t[:].offset,
                                      [ps2t[:].ap[0], [512, 2], [128, 4], [1, 128]])
                        dst = bass.AP(oh[:].tensor, oh[:].offset,
                                      [oh[:].ap[0], [512, 2], [1, 4], [4, 128]])
                        ev += 1
                        if ev % 4 == 0:
                            nc.vector.tensor_copy(dst, src)
                        else:
                            nc.scalar.copy(dst, src)
                        nc.sync.dma_start(
                            outd[i, :, half * 1024:(half + 1) * 1024], oh[:])

            if TIMING_MODE:
                nc.sync.dma_start(tiny[:, :], consts[0:1, 0:64].bitcast(F32))

    nc.compile()
    return nc


_NC = None


def _get_nc():
    global _NC
    if _NC is None:
        _NC = _build()
    return _NC


def kernel(x: np.ndarray) -> np.ndarray:
    x = np.asarray(x, dtype=np.float32)
    assert x.shape == (8, 1, 512, 512), x.shape
    nc = _get_nc()
    cinh = _dft_consts()
    in_maps = [
        {"xin": np.ascontiguousarray(x[b, 0].reshape(128, 2048)).astype(np.float16),
         "cinh": cinh}
        for b in range(8)
    ]
    res = run_bass_kernel_spmd(nc, in_maps, core_ids=list(range(8)))
    out = np.stack([r["outd"].reshape(C, 512, 512).astype(np.float32)
                    for r in res.results])
    return out


if __name__ == "__main__":
    rng = np.random.RandomState(0)
    x = rng.randn(8, 1, 512, 512).astype(np.float32)
    y = kernel(x)
    print(y.shape, y.dtype)


# revision 11
# speedup vs baseline: 1.1287x; 1.1287x over previous
"""Trainium2 Bass kernel for nn_FT_init: pixel_unshuffle -> FFT2 -> all-pairs
magnitude/phase recombination -> IFFT2 -> pixel_shuffle.

Strategy: pure data parallel, one sample per NeuronCore (8 cores).

All matmul operands are fp16 (1 cycle/row at any moving width + fast weight
load eligibility; ~5e-4 per-rounding error vs the 2e-2 gate).  PSUM
accumulation stays fp32.

Forward FFT2: stage A streams 256-col [Wr|Wi] movs per channel; stage B is
half-width packed — only frequency cols n = 0..64 of Z are ever consumed
downstream (every recombined spectrum is Hermitian), so stage B computes
[Zr(0:65)|Zi(0:65)] with 130-col movs and the whole magnitude / unit-phase
pipeline runs at half width.

Inverse (Hermitian-folded): for channel pair (i, j) the spectrum
c = mag_i * u_j is Hermitian, so its IFFT2 is real and stage 1 only needs
packed output rows [S1r(n=0..64) | S1i(n=1..63)] = exactly 128 PSUM
partitions from two 128-col matmuls (stationary = packed c tiles, movings =
Re/-Im IDFT consts).  Stage 2 is a single 128-col matmul per j with one
shared folded const; the pixel-shuffle interleave happens in the PSUM
eviction AP.  PE streaming: 384 cols per (i, j) unit vs 1024 per j-pair in
the f32r scheme.  Evictions are [128,1024] two-bank ops to amortize per-op
overhead; output goes to DRAM as fp16 and is upcast on host.
"""
import sys

sys.path.insert(0, "/opt/trn_rl_repo")

import numpy as np
import concourse.bass as bass  # noqa: E402
import concourse.mybir as mybir  # noqa: E402
import concourse.tile as tile  # noqa: E402
import concourse.bacc as bacc  # noqa: E402
from concourse.bass_utils import run_bass_kernel_spmd  # noqa: E402

F32 = mybir.dt.float32
FP16 = mybir.dt.float16
N = 128  # DFT size
R = 4  # msfa / pixel shuffle factor
C = 16  # channels = R*R
MULT = mybir.AluOpType.mult
ADD = mybir.AluOpType.add
SQRT = mybir.ActivationFunctionType.Sqrt


def _dft_consts():
    k = np.arange(N)
    ang = 2.0 * np.pi / N * np.outer(k, k)
    Wr = np.cos(ang).astype(np.float32)
    Wi = (-np.sin(ang)).astype(np.float32)
    Er = (np.cos(ang) / N).astype(np.float32)
    Ei = (np.sin(ang) / N).astype(np.float32)
    # stage-2 folded const: out[y,x] = sum_k s[k,y] * M2[k,x] with
    # s = [S1r(n=0..64) | S1i(n=1..63)]
    w = np.ones(65, np.float32)
    w[1:64] = 2.0
    M2 = np.zeros((N, N), np.float32)
    M2[0:65] = w[:, None] * Er[0:65]
    M2[65:128] = -2.0 * Ei[1:64]
    cinh = np.hstack([
        Wr, Wi,                      # WWh      [0:256]
        Wr[:, 0:65], Wi[:, 0:65],    # WB1      [256:386]
        -Wi[:, 0:65], Wr[:, 0:65],   # WB2      [386:516]
        Er,                          # Erh      [516:644]
        -Ei,                         # Ginh     [644:772]
        M2,                          # M2h      [772:900]
    ]).astype(np.float16)
    return cinh


REPEAT = 1  # >1 only for timing experiments
TIMING_MODE = False  # True: big output stays on-device (for wall-time diffs)


def _build():
    nc = bacc.Bacc("TRN2", target_bir_lowering=False, debug=False, num_devices=8)
    xin = nc.dram_tensor("xin", [128, 2048], FP16, kind="ExternalInput")
    cinh = nc.dram_tensor("cinh", [128, 900], FP16, kind="ExternalInput")
    if TIMING_MODE:
        outd = nc.dram_tensor("outd", [C, 128, 2048], FP16, kind="Internal")
        tiny = nc.dram_tensor("tiny", [1, 128], F32, kind="ExternalOutput")
    else:
        outd = nc.dram_tensor("outd", [C, 128, 2048], FP16, kind="ExternalOutput")

    with tile.TileContext(nc) as tc:
        with (
            tc.tile_pool(name="persist", bufs=1) as pp,
            tc.tile_pool(name="ew", bufs=2) as ew,
            tc.tile_pool(name="sx", bufs=3) as sxp,
            tc.tile_pool(name="cpool", bufs=4) as cpool,
            tc.tile_pool(name="s4pool", bufs=4) as s4p,
            tc.tile_pool(name="ohpool", bufs=4) as ohp,
            tc.tile_pool(name="ps1", bufs=2, space="PSUM") as ps1,
            tc.tile_pool(name="ps2", bufs=2, space="PSUM") as ps2,
        ):
            consts = pp.tile([128, 900], FP16)
            nc.sync.dma_start(consts[:], cinh[:, :])
            WWh = consts[:, 0:256]
            WB1 = consts[:, 256:386]
            WB2 = consts[:, 386:516]
            Erh = consts[:, 516:644]
            Ginh = consts[:, 644:772]
            M2h = consts[:, 772:900]

            xrows = pp.tile([128, 2048], FP16)
            nc.sync.dma_start(xrows[:], xin[:, :])

            for _rep in range(REPEAT):
                # zs: per channel c, 130 cols: [zr(n=0..64) | zi(n=0..64)]
                zs = pp.tile([128, 2080], F32)

                # ---- forward FFT2 (fp16), 2 channels per group ----
                # software-pipelined: stage-A MMs of group g+1 issue before
                # stage-B MMs of group g so the PE streams through the
                # sx-eviction gaps.
                psA_t = [None] * 8
                sx_t = [None] * 8

                def fwd_A(g):
                    psA = ps1.tile([128, 512], F32, tag="s1")
                    psA_t[g] = psA
                    for cc in range(2):
                        c = g * 2 + cc
                        p, q = divmod(c, R)
                        xs = bass.AP(
                            xrows[:].tensor,
                            xrows[:].offset + p * 512 + q,
                            [xrows[:].ap[0], [4, 128]],
                        )
                        nc.tensor.matmul(psA[:, cc * 256:(cc + 1) * 256], xs, WWh,
                                         start=True, stop=True)
                    sx = sxp.tile([128, 512], FP16)
                    sx_t[g] = sx
                    if g % 2 == 0:
                        nc.scalar.copy(sx[:], psA[:])
                    else:
                        nc.vector.tensor_copy(sx[:], psA[:])

                def fwd_B(g):
                    sx = sx_t[g]
                    psB = ps2.tile([128, 512], F32, tag="s2")
                    for cc in range(2):
                        o = cc * 256
                        zo = cc * 130
                        nc.tensor.matmul(psB[:, zo:zo + 130], sx[:, o:o + 128],
                                         WB1, start=True, stop=False)
                        nc.tensor.matmul(psB[:, zo:zo + 130], sx[:, o + 128:o + 256],
                                         WB2, start=False, stop=True)
                    # evict z: [Zr(0:65)|Zi(0:65)] x 2 chans, contiguous 260
                    if g % 2 == 0:
                        nc.vector.tensor_copy(zs[:, g * 260:(g + 1) * 260],
                                              psB[:, 0:260])
                    else:
                        nc.scalar.copy(zs[:, g * 260:(g + 1) * 260],
                                       psB[:, 0:260])

                for g in range(8):
                    fwd_A(g)
                    if g >= 1:
                        fwd_B(g - 1)
                fwd_B(7)

                # ---- magnitude / unit phase, half width (cols 0..64) ----
                # processed in 2 halves of 8 channels each so the inverse can
                # start while the second half is still in flight.
                t1 = pp.tile([128, 1040], F32)
                t2 = pp.tile([128, 1040], F32)
                sq = pp.tile([128, 1040], F32)
                mag_f = pp.tile([128, 1040], F32)
                rmag = pp.tile([128, 1040], F32)
                rmagn = pp.tile([128, 1040], F32)
                uPA = pp.tile([128, 2048], FP16)
                uPB = pp.tile([128, 2048], FP16)
                magP = pp.tile([128, 2048], FP16)

                def apv(t, off, dims):
                    return bass.AP(t[:].tensor, t[:].offset + off, [t[:].ap[0]] + dims)

                for h in range(2):
                    zo = h * 1040  # 8 chans * 130
                    co = h * 520   # 8 chans * 65
                    uo = h * 1024
                    zr_v = apv(zs, zo, [[130, 8], [1, 65]])
                    zi_v = apv(zs, zo + 65, [[130, 8], [1, 65]])
                    t1v = apv(t1, co, [[65, 8], [1, 65]])
                    t2v = apv(t2, co, [[65, 8], [1, 65]])
                    nc.vector.tensor_tensor(t1v, zr_v, zr_v, MULT)
                    nc.vector.tensor_tensor(t2v, zi_v, zi_v, MULT)
                    nc.vector.tensor_tensor(sq[:, co:co + 520], t1[:, co:co + 520],
                                            t2[:, co:co + 520], ADD)
                    nc.scalar.activation(mag_f[:, co:co + 520], sq[:, co:co + 520],
                                         SQRT)
                    scr = ew.tile([128, 520], F32, tag="scr")
                    nc.vector.reciprocal_approx_accurate(
                        rmag[:, co:co + 520], mag_f[:, co:co + 520], scr[:])
                    nc.vector.tensor_scalar_mul(rmagn[:, co:co + 520],
                                                rmag[:, co:co + 520], -1.0)
                    # packed unit-phase tiles (fp16):
                    # uPA[j-block] = [ur(0:65) | ui(1:64)]
                    # uPB[j-block] = [ui(0:65) | -ur(1:64)]
                    nc.vector.tensor_tensor(
                        apv(uPA, uo, [[128, 8], [1, 65]]), zr_v,
                        apv(rmag, co, [[65, 8], [1, 65]]), MULT)
                    nc.vector.tensor_tensor(
                        apv(uPA, uo + 65, [[128, 8], [1, 63]]),
                        apv(zs, zo + 66, [[130, 8], [1, 63]]),
                        apv(rmag, co + 1, [[65, 8], [1, 63]]), MULT)
                    nc.vector.tensor_tensor(
                        apv(uPB, uo, [[128, 8], [1, 65]]), zi_v,
                        apv(rmag, co, [[65, 8], [1, 65]]), MULT)
                    nc.vector.tensor_tensor(
                        apv(uPB, uo + 65, [[128, 8], [1, 63]]),
                        apv(zs, zo + 1, [[130, 8], [1, 63]]),
                        apv(rmagn, co + 1, [[65, 8], [1, 63]]), MULT)
                    # packed magnitudes per i: [mag(0:65) | mag(1:64)]
                    nc.vector.tensor_copy(
                        apv(magP, uo, [[128, 8], [1, 65]]),
                        apv(mag_f, co, [[65, 8], [1, 65]]))
                    nc.vector.tensor_copy(
                        apv(magP, uo + 65, [[128, 8], [1, 63]]),
                        apv(mag_f, co + 1, [[65, 8], [1, 63]]))

                # ---- inverse: per magnitude channel i ----
                ev = 0
                for i in range(C):
                    mb = bass.AP(magP[:].tensor, magP[:].offset + i * 128,
                                 [magP[:].ap[0], [0, 8], [1, 128]])
                    cA = cpool.tile([128, 2048], FP16, tag="cA")
                    cB = cpool.tile([128, 2048], FP16, tag="cB")

                    for half in range(2):
                        uo = half * 1024
                        cAv = apv(cA, uo, [[128, 8], [1, 128]])
                        cBv = apv(cB, uo, [[128, 8], [1, 128]])
                        uAv = apv(uPA, uo, [[128, 8], [1, 128]])
                        uBv = apv(uPB, uo, [[128, 8], [1, 128]])
                        if i % 8 == 7:
                            nc.gpsimd.tensor_tensor(cAv, uAv, mb, MULT)
                        else:
                            nc.vector.tensor_tensor(cAv, uAv, mb, MULT)
                        if i % 2 == 1:
                            nc.gpsimd.tensor_tensor(cBv, uBv, mb, MULT)
                        else:
                            nc.vector.tensor_tensor(cBv, uBv, mb, MULT)
                        ps1t = ps1.tile([128, 1024], F32, tag="s1")
                        for jj in range(8):
                            j = half * 8 + jj
                            o = jj * 128
                            nc.tensor.matmul(ps1t[:, o:o + 128],
                                             cA[:, j * 128:(j + 1) * 128], Erh,
                                             start=True, stop=False)
                            nc.tensor.matmul(ps1t[:, o:o + 128],
                                             cB[:, j * 128:(j + 1) * 128], Ginh,
                                             start=False, stop=True)
                        s4 = s4p.tile([128, 1024], FP16)
                        ev += 1
                        if ev % 4 == 0:
                            nc.vector.tensor_copy(s4[:], ps1t[:])
                        else:
                            nc.scalar.copy(s4[:], ps1t[:])

                        ps2t = ps2.tile([128, 1024], F32, tag="s2")
                        for jj in range(8):
                            o = jj * 128
                            nc.tensor.matmul(ps2t[:, o:o + 128],
                                             s4[:, jj * 128:(jj + 1) * 128],
                                             M2h, start=True, stop=True)
                        # evict with pixel-shuffle interleave:
                        # oh[:, pp*512 + 4x + jx] = ps2t[:, (pp*4+jx)*128 + x]
                        oh = ohp.tile([128, 1024], FP16)
                        src = bass.AP(ps2t[:].tensor, ps2t[:].offset,
                                      [ps2t[:].ap[0], [512, 2], [128, 4], [1, 128]])
                        dst = bass.AP(oh[:].tensor, oh[:].offset,
                                      [oh[:].ap[0], [512, 2], [1, 4], [4, 128]])
                        ev += 1
                        if ev % 4 == 0:
                            nc.vector.tensor_copy(dst, src)
                        else:
                            nc.scalar.copy(dst, src)
                        nc.sync.dma_start(
                            outd[i, :, half * 1024:(half + 1) * 1024], oh[:])

            if TIMING_MODE:
                nc.sync.dma_start(tiny[:, :], consts[0:1, 0:64].bitcast(F32))

    nc.compile()
    return nc


_NC = None


def _get_nc():
    global _NC
    if _NC is None:
        _NC = _build()
    return _NC


def kernel(x: np.ndarray) -> np.ndarray:
    x = np.asarray(x, dtype=np.float32)
    assert x.shape == (8, 1, 512, 512), x.shape
    nc = _get_nc()
    cinh = _dft_consts()
    in_maps = [
        {"xin": np.ascontiguousarray(x[b, 0].reshape(128, 2048)).astype(np.float16),
         "cinh": cinh}
        for b in range(8)
    ]
    res = run_bass_kernel_spmd(nc, in_maps, core_ids=list(range(8)))
    out = np.stack([r["outd"].reshape(C, 512, 512).astype(np.float32)
                    for r in res.results])
    return out


if __name__ == "__main__":
    rng = np.random.RandomState(0)
    x = rng.randn(8, 1, 512, 512).astype(np.float32)
    y = kernel(x)
    print(y.shape, y.dtype)
